# revision 57
# baseline (speedup 1.0000x reference)
"""Trainium2 Bass kernel for a GQA sliding-window attention layer.

Reference computation (B=2, T=2048, C=2048, 16 Q heads / 4 KV heads, d=128):
    q = x @ Wq; k = x @ Wk; v = x @ Wv (+ sigmoid-gated value embedding)
    q, k = rmsnorm(rope(q)), rmsnorm(rope(k))
    scores masked to the band 0 <= j - i < window (=1024), softmax over j
    out = (p @ v) @ Wo

Sharding: 8 cores = 2 batches x 4 KV groups.  Each core computes its 4 Q
heads / 1 KV head for one batch and a partial output (its 512-row slice of
the Wo contraction); the host sums the 4 partials per batch (bf16 partials).

v2 changes vs the 377us baseline (all aimed at the PE bottleneck):
  - softmax denominators no longer use 108 ones-matmuls: exp tiles are
    accumulated on DVE/Pool (bf16 chains), then ONE ones[128,128] matmul
    per q-tile both sums over kj and broadcasts 1/den across partitions.
  - band masks are multiplicative 0/1 masks applied on Pool after exp
    (removes 24 PE bias-matmuls).
  - rms row broadcasts are free: the sum-of-squares matmul uses a
    ones[128,128] stationary, so every partition already has the row sum
    (removes 36 broadcast matmuls).
  - projections emitted in waves that reuse one stationary for 2-4
    matmuls; out-proj is h-outer/co-inner; a post-schedule IR pass
    deletes redundant InstLdweights (same weights AP, no sync info).
  - output written in bf16 (halves the 16MB/core output DMA).
"""

import numpy as np
import ml_dtypes
from collections import deque

BF16 = ml_dtypes.bfloat16

B, T, C = 2, 2048, 2048
N_HEAD, N_KV, HD, GATE_CH = 16, 4, 128, 32
WINDOW = 1024
P = 128
GH = N_HEAD // N_KV  # q heads per kv head (= per core)
N_CORES = 8

_PROGRAM_CACHE = {}


def dedup_ldweights(nc):
    """Remove InstLdweights that reload the stationary already in the PE.

    Safe subset only: the candidate must have identical (memref, offset,
    pattern, dtype) to the previous PE weight load in the same block, and
    carry no semaphore waits/updates (sync_info None).  Content safety:
    between two identical loads with no other InstLdweights in between, no
    producer rewrites the weights region (weight tiles are write-once per
    buffer rotation, and any rotation reuse has many other loads between).
    """

    def sig(inst):
        ap = inst.ins[0]
        try:
            return (ap.memref, ap.offset, str(ap.ap), str(ap.dtype))
        except Exception:
            return None

    removed = 0
    for f in nc.m.functions:
        for b in f.blocks:
            il = b.instructions
            last = None
            doomed = []
            for inst in il:
                tn = type(inst).__name__
                if tn == "InstLdweights":
                    s = sig(inst)
                    if s is not None and s == last and inst.sync_info is None:
                        doomed.append(inst)
                    else:
                        last = s
            for inst in doomed:
                il.remove(inst)
                removed += 1
    return removed


def build_program(T_=T, C_=C, win=WINDOW, dedup=True):
    import concourse.mybir as mybir
    import concourse.tile as tile
    from concourse import bacc

    dt = mybir.dt
    f32 = dt.float32
    bf16 = dt.bfloat16
    AF = mybir.ActivationFunctionType
    ALU = mybir.AluOpType

    NT = T_ // P          # token tiles
    KT = C_ // P          # contraction tiles
    WT = win // P         # window tiles
    TS = T_ // 512        # 512-wide token slices

    nc = bacc.Bacc()

    # every input is host-pre-arranged to a partition-major [128, N] image
    # so each DMA is one contiguous blast (per-DMA queue overhead is >1us,
    # and partition-strided gathers process one DRAM row at a time)
    xT = nc.declare_dram_parameter("xT", [P, KT, T_], bf16, isOutput=False)
    wq = nc.declare_dram_parameter("wq", [P, GH, KT, HD], bf16, isOutput=False)
    wk = nc.declare_dram_parameter("wk", [P, KT, HD], bf16, isOutput=False)
    wv = nc.declare_dram_parameter("wv", [P, KT, HD], bf16, isOutput=False)
    wg = nc.declare_dram_parameter("wg", [GATE_CH, 1], bf16, isOutput=False)
    ve2 = nc.declare_dram_parameter("ve2", [P, NT, HD], bf16, isOutput=False)
    wo = nc.declare_dram_parameter("wo", [P, GH, C_], bf16, isOutput=False)
    ccd = nc.declare_dram_parameter("cc", [P, T_], bf16, isOutput=False)
    ssd = nc.declare_dram_parameter("ss", [P, T_], bf16, isOutput=False)
    mlo = nc.declare_dram_parameter("mlo", [P, GH * P], bf16, isOutput=False)
    mhi = nc.declare_dram_parameter("mhi", [P, GH * P], bf16, isOutput=False)
    idb = nc.declare_dram_parameter("identb", [P, P], bf16, isOutput=False)
    out_d = nc.declare_dram_parameter("out", [T_, C_], bf16, isOutput=True)

    with tile.TileContext(nc) as tc:
        with (
            tc.tile_pool(name="singles", bufs=1) as sg,
            tc.tile_pool(name="rope", bufs=3) as rp,
            tc.tile_pool(name="rms", bufs=2) as rm,
            tc.tile_pool(name="attw", bufs=7) as aw,
            tc.tile_pool(name="accs", bufs=2) as acp,
            tc.tile_pool(name="yqp", bufs=2) as yqp,
            tc.tile_pool(name="outp", bufs=4) as op_pool,
            tc.tile_pool(name="psA", bufs=3, space="PSUM") as ppA,   # wavesA / scores (3)
            tc.tile_pool(name="psY", bufs=3, space="PSUM") as ppY,   # wavesB / yps+den (3)
            tc.tile_pool(name="psO", bufs=2, space="PSUM") as ppO,   # tails / out pairs (2)
        ):
            # ---- input DMAs ------------------------------------------
            # critical path (sync queue): wk, q0, then xT in 4 quads the
            # first waves chase, with the other q heads interleaved.
            # everything wave3+ needs rides the scalar engine's hw DMA
            # queue in parallel (bandwidth-shared, queue-independent).
            # HBM read bandwidth is the wall (~320GB/s aggregate across
            # queues), so priority = bytes-needed-first: wk/q0 + the four
            # x quads split across sync/scalar; everything later-needed
            # (q1-q3, rope tables, v/ve2/wo, masks) rides the gpsimd
            # software-DGE queue behind them.
            wq_sb = sg.tile([P, GH, KT, HD], bf16, tag="wq")
            wk_sb = sg.tile([P, KT, HD], bf16, tag="wk")
            wv_sb = sg.tile([P, KT, HD], bf16, tag="wv")
            nc.sync.dma_start(out=wk_sb[:], in_=wk[:])
            nc.scalar.dma_start(out=wq_sb[:, 0], in_=wq[:, 0])
            xq = []
            for q4 in range(4):
                t_ = sg.tile([P, 4, T_], bf16, tag=f"xq{q4}")
                eng = nc.sync if q4 % 2 == 0 else nc.scalar
                eng.dma_start(out=t_[:], in_=xT[:, 4 * q4:4 * (q4 + 1), :])
                xq.append(t_)
            xt = [xq[kt // 4][:, kt % 4, :] for kt in range(KT)]
            cc_sb = sg.tile([P, T_], bf16, tag="cc")
            nc.sync.dma_start(out=cc_sb[:], in_=ccd[:])
            ss_sb = sg.tile([P, T_], bf16, tag="ss")
            nc.scalar.dma_start(out=ss_sb[:], in_=ssd[:])
            wg_sb = sg.tile([GATE_CH, 1], bf16, tag="wg")
            nc.gpsimd.dma_start(out=wg_sb[:], in_=wg[:])
            idb_sb = sg.tile([P, P], bf16, tag="idb")
            nc.gpsimd.dma_start(out=idb_sb[:], in_=idb[:])
            nc.gpsimd.dma_start(out=wq_sb[:, 1], in_=wq[:, 1])
            nc.gpsimd.dma_start(out=wq_sb[:, 2], in_=wq[:, 2])
            nc.gpsimd.dma_start(out=wq_sb[:, 3], in_=wq[:, 3])
            mlo_sb = sg.tile([P, GH * P], bf16, tag="mlo")
            nc.gpsimd.dma_start(out=mlo_sb[:], in_=mlo[:])
            mhi_sb = sg.tile([P, GH * P], bf16, tag="mhi")
            nc.gpsimd.dma_start(out=mhi_sb[:], in_=mhi[:])
            nc.gpsimd.dma_start(out=wv_sb[:], in_=wv[:])
            ve2_sb = sg.tile([P, NT, HD], bf16, tag="ve2")
            nc.gpsimd.dma_start(out=ve2_sb[:], in_=ve2[:])
            wo_sb = sg.tile([P, GH, C_], bf16, tag="wo")
            nc.gpsimd.dma_start(out=wo_sb[:], in_=wo[:])

            ones_sb = sg.tile([P, P], bf16, tag="ones128")
            nc.vector.memset(ones_sb[:], 1.0)
            epsq_sb = sg.tile([P, 1], f32, tag="epsq")
            nc.vector.memset(epsq_sb[:], 1e-6)
            epsk_sb = sg.tile([P, 1], f32, tag="epsk")
            nc.vector.memset(epsk_sb[:], HD * 1e-6)

            # persistent intermediates
            qhat = sg.tile([P, GH, T_], bf16, tag="qhat")   # normalized roped q, [d, h, t]
            khat = sg.tile([P, T_], bf16, tag="khat")       # normalized roped k * isq
            vsb = sg.tile([P, NT, HD], bf16, tag="vsb")     # gated v, [tok, tt, d]

            # ---- projections: double-buffered waves of 3 ---------------
            # head ids: 0=k, 1=v, 2..5=q0..q3.  Waves alternate between
            # two 3-bank psum pools, so a new wave's kt=0 matmuls never
            # wait for the previous wave's tails to free banks.  The v
            # waves run LAST: their tails have no DVE rope work, so the
            # DVE enters attention with an empty queue (the q-rope tails
            # all overlap earlier waves' matmuls).
            WAVES = [
                [(0, 0), (0, 1), (0, 2)],
                [(0, 3), (2, 0), (2, 1)],
                [(3, 0), (3, 1), (4, 0)],
                [(4, 1), (5, 0), (5, 1)],
                [(2, 2), (2, 3), (3, 2)],
                [(3, 3), (4, 2), (4, 3)],
                [(5, 2), (5, 3), (1, 0)],
                [(1, 1), (1, 2), (1, 3)],
            ]

            def w_ap(head, kt):
                if head == 0:
                    return wk_sb[:, kt, :]
                if head == 1:
                    return wv_sb[:, kt, :]
                return wq_sb[:, head - 2, kt, :]

            def wave_alloc(wi):
                pool_, tag_ = (ppA, "sc") if wi % 2 == 0 else (ppY, "yp")
                items = []
                for (head, ts_) in WAVES[wi]:
                    sl = slice(ts_ * 512, ts_ * 512 + 512)
                    ps = pool_.tile([P, 512], f32, tag=tag_, name=f"ps{head}_{ts_}")
                    items.append((head, sl, ps))
                return items

            def wave_kt(items, kt):
                # group by head so consecutive matmuls share one stationary
                for (head, sl, ps) in items:
                    nc.tensor.matmul(
                        ps[:], lhsT=w_ap(head, kt), rhs=xt[kt][:, sl],
                        start=(kt == 0), stop=(kt == KT - 1),
                    )

            def wave_mms(wi):
                items = wave_alloc(wi)
                for kt in range(KT):
                    wave_kt(items, kt)
                return items

            def v_tail(sl, ps):
                # vT psum [d, tok] -> bf16 sbuf, PE-transpose each 128-tok
                # block to [tok, d], add sigmoid-gated ve.  sigmoid is
                # computed as 1/(1+exp(-g)) so the ACT engine only ever
                # needs the exp/ln table (no activation-table thrash).
                vt = rp.tile([P, 512], bf16, tag="vt")
                nc.scalar.copy(out=vt[:], in_=ps[:])
                for i in range(4):
                    tt = sl.start // P + i
                    tsl = slice(tt * P, (tt + 1) * P)
                    tp = ppO.tile([P, P], bf16, tag="op", name=f"tp{tt}")
                    nc.tensor.transpose(tp[:], vt[:, i * P:(i + 1) * P], idb_sb[:])
                    gps = ppO.tile([P, 1], f32, tag="op", name=f"gp{tt}")
                    nc.tensor.matmul(gps[:], lhsT=xt[0][0:GATE_CH, tsl],
                                     rhs=wg_sb[:], start=True, stop=True)
                    gexp = rp.tile([P, 1], f32, tag="gexp")
                    nc.scalar.activation(gexp[:], gps[:], AF.Exp, scale=-1.0)
                    gp1 = rp.tile([P, 1], f32, tag="gp1")
                    nc.vector.tensor_scalar_add(gp1[:], gexp[:], 1.0)
                    gcol = rp.tile([P, 1], f32, tag="gcol")
                    nc.vector.reciprocal_approx_fast(gcol[:], gp1[:])
                    # v = ve2 * sigmoid(g) + v_proj (ve2 pre-scaled by 2)
                    nc.vector.scalar_tensor_tensor(
                        out=vsb[:, tt, :], in0=ve2_sb[:, tt, :], scalar=gcol[:],
                        in1=tp[:], op0=ALU.mult, op1=ALU.add,
                    )

            def qk_tail(head, sl, ps):
                # rope: qr = ps*cc + swap(ps)*ss  (ss carries the sign).
                # engine split: DVE does 3 ops, Pool 1, ACT 1 — DVE was the
                # projection-phase bottleneck when it carried all of rope.
                qr = rp.tile([P, 512], f32, tag="qr")
                nc.vector.tensor_mul(qr[:], ps[:], cc_sb[:, sl])
                qs = rp.tile([P, 512], f32, tag="qs")
                nc.vector.tensor_mul(qs[0:64, :], ps[64:128, :], ss_sb[0:64, sl])
                nc.vector.tensor_mul(qs[64:128, :], ps[0:64, :], ss_sb[64:128, sl])
                nc.vector.tensor_add(qr[:], qr[:], qs[:])
                q2 = rp.tile([P, 512], bf16, tag="q2")
                nc.scalar.square(q2[:], qr[:])
                # ssq with ones[128,128] stationary: every partition gets the
                # column sum -> rms row already broadcast, no extra matmul.
                ssq = ppO.tile([P, 512], f32, tag="op",
                               name=f"ssq{head}_{sl.start}")
                nc.tensor.matmul(ssq[:], lhsT=ones_sb[:], rhs=q2[:],
                                 start=True, stop=True)
                # Sqrt thrashes the ACT table vs Exp, but with the v waves
                # last ALL sqrts complete before the first attention exp,
                # so the kernel pays only one swap.
                srow = rm.tile([P, 512], f32, tag="srow")
                if head == 0:
                    # k: fold the 1/sqrt(d) score scale into k-hat:
                    # 1/sqrt(ssq + HD*eps) = isq/sqrt(ms + eps)
                    nc.scalar.activation(srow[:], ssq[:], AF.Sqrt,
                                         bias=epsk_sb[:], scale=1.0)
                else:
                    nc.scalar.activation(srow[:], ssq[:], AF.Sqrt,
                                         bias=epsq_sb[:], scale=1.0 / HD)
                rr = rm.tile([P, 512], f32, tag="rr")
                nc.vector.reciprocal_approx_fast(rr[:], srow[:])
                dest = khat[:, sl] if head == 0 else qhat[:, head - 2, sl]
                nc.gpsimd.tensor_mul(dest, qr[:], rr[:])

            def wave_tails(items):
                for (head, sl, ps) in items:
                    if head == 1:
                        v_tail(sl, ps)
                    else:
                        qk_tail(head, sl, ps)

            # waves 1+2 are emitted kt-interleaved: during the xT DMA
            # chase the PE has 6 matmuls per arriving tile instead of 3.
            it0, it1 = wave_alloc(0), wave_alloc(1)
            for kt in range(KT):
                wave_kt(it0, kt)
                wave_kt(it1, kt)
            wave_tails(it0)
            prev_items = it1
            for wi in range(2, len(WAVES)):
                items = wave_mms(wi)
                wave_tails(prev_items)
                prev_items = items
            last_wave_items = prev_items

            # the last waves are the v projections: their tails carry no
            # DVE rope work, so attention starts with clean DVE/Pool
            # queues.  qi=0's scores are emitted BEFORE the last v wave's
            # tails so the exp pipeline warms up while the PE finishes the
            # v transposes (its PVs only pop later, when vsb is ready).

            # ---- attention + out-proj ---------------------------------
            # S^T tiles [kj, (h,q)] fused across the 4 heads (512 wide).
            CO = C_ // 512
            ISQ_NONE = None  # isq folded into khat

            yps = {}
            accA = {}
            accB = {}
            accb = {}
            denp = {}
            rds = {}
            yqs = {}

            def ktc_of(qi):
                return min(WT + 1, NT - qi)

            def emit_scores(qi, kk):
                kt = qi + kk
                qs4 = qhat[:, :, qi * P:(qi + 1) * P]
                sp = ppA.tile([P, GH * P], f32, tag="sc", name=f"sp{qi}_{kk}")
                nc.tensor.matmul(sp[:], lhsT=khat[:, kt * P:(kt + 1) * P],
                                 rhs=qs4, start=True, stop=True)
                pt = aw.tile([P, GH * P], bf16, tag="pt")
                nc.scalar.activation(pt[:], sp[:], AF.Exp)
                masked_lo = (kk == 0)
                masked_hi = (kk == WT and ktc_of(qi) == WT + 1)
                if masked_lo or masked_hi:
                    ptm = aw.tile([P, GH * P], bf16, tag="pt")
                    nc.gpsimd.tensor_mul(
                        ptm[:], pt[:], mlo_sb[:] if masked_lo else mhi_sb[:])
                    pt = ptm
                return pt

            # first few q-tiles compute den on the PE (ones-matmul per kk):
            # at attention start DVE/Pool are still draining the last
            # projection tails, so the bf16 chains would stall the den.
            PE_DEN = frozenset(range(2))

            def emit_pv(qi, kk, pt):
                ktc = ktc_of(qi)
                kt = qi + kk
                if kk == 0:
                    yps[qi] = ppY.tile([P, GH * P], f32, tag="yp", name=f"yp{qi}")
                if qi in PE_DEN:
                    if kk == 0:
                        denp[qi] = ppY.tile([P, GH * P], f32, tag="yp",
                                            name=f"dn{qi}")
                    nc.tensor.matmul(
                        denp[qi][:], lhsT=ones_sb[:], rhs=pt[:],
                        start=(kk == 0), stop=(kk == ktc - 1),
                    )
                elif kk == 0:
                    accA[qi] = pt   # bf16 partial-sum chain heads
                elif kk == 1:
                    accB[qi] = pt
                elif kk % 2 == 0:
                    if kk == 2:
                        t_ = acp.tile([P, GH * P], bf16, tag="accA")
                        nc.vector.tensor_add(t_[:], accA[qi][:], pt[:])
                        accA[qi] = t_
                    else:
                        nc.vector.tensor_add(accA[qi][:], accA[qi][:], pt[:])
                else:
                    if kk == 3:
                        t_ = acp.tile([P, GH * P], bf16, tag="accB")
                        nc.gpsimd.tensor_add(t_[:], accB[qi][:], pt[:])
                        accB[qi] = t_
                    else:
                        nc.gpsimd.tensor_add(accB[qi][:], accB[qi][:], pt[:])
                nc.tensor.matmul(
                    yps[qi][:], lhsT=vsb[:, kt, :], rhs=pt[:],
                    start=(kk == 0), stop=(kk == ktc - 1),
                )

            def emit_merge(qi):
                if qi in PE_DEN:
                    return
                if qi not in accB:
                    accb[qi] = accA[qi]
                else:
                    t_ = acp.tile([P, GH * P], bf16, tag="accM")
                    nc.vector.tensor_add(t_[:], accA[qi][:], accB[qi][:])
                    accb[qi] = t_

            def emit_den_yq(qi):
                if qi not in PE_DEN:
                    # one matmul: sums over kj AND broadcasts across partitions
                    denp[qi] = ppY.tile([P, GH * P], f32, tag="yp",
                                        name=f"dn{qi}")
                    nc.tensor.matmul(denp[qi][:], lhsT=ones_sb[:],
                                     rhs=accb[qi][:], start=True, stop=True)
                rd = rm.tile([P, GH * P], f32, tag="rd")
                nc.vector.reciprocal_approx_fast(rd[:], denp[qi][:])
                rds[qi] = rd
                yq = yqp.tile([P, GH * P], bf16, tag="yq")
                nc.vector.tensor_mul(yq[:], yps[qi][:], rds[qi][:])
                yqs[qi] = yq

            def emit_out(qi, half):
                # one half = 2 adjacent psO banks: 8 matmuls (h-outer so
                # LDWEIGHTS dedups), one paired [P,1024] DVE copy, one DMA
                qsl = slice(qi * P, (qi + 1) * P)
                yq = yqs[qi]
                o0 = ppO.tile([P, 512], f32, tag="op", name=f"op{qi}_{half}a")
                o1 = ppO.tile([P, 512], f32, tag="op", name=f"op{qi}_{half}b")
                for h in range(GH):
                    for co, ops_ in ((2 * half, o0), (2 * half + 1, o1)):
                        nc.tensor.matmul(
                            ops_[:], lhsT=yq[:, h * P:(h + 1) * P],
                            rhs=wo_sb[:, h, co * 512:(co + 1) * 512],
                            start=(h == 0), stop=(h == GH - 1),
                        )
                ob = op_pool.tile([P, 1024], bf16, tag="ob")
                nc.vector.tensor_copy(out=ob[:, 0:512], in_=o0[:])
                nc.vector.tensor_copy(out=ob[:, 512:1024], in_=o1[:])
                nc.sync.dma_start(
                    out=out_d[qsl, half * 1024:(half + 1) * 1024], in_=ob[:])

            # main interleaved loop: scores run 4 ahead of PV (gives the
            # exp 4 PE-steps of slack); the deferred den/yq/out stages of
            # qi run spread through qi+1's stream so the PE never waits on
            # the DVE/ACT chains.
            PV_DELAY = 4

            def finish_pv(q_, k_, p_):
                emit_pv(q_, k_, p_)
                if k_ == ktc_of(q_) - 1:
                    emit_merge(q_)
                    deferred.append(lambda q=q_: emit_den_yq(q))
                    deferred.append(lambda q=q_: emit_out(q, 0))
                    deferred.append(lambda q=q_: emit_out(q, 1))

            pv_queue = deque()
            deferred = deque()

            def att_qi(qi, tail_hook=None):
                ktc = ktc_of(qi)
                for kk in range(ktc):
                    if tail_hook is not None and kk == PV_DELAY:
                        # last v-wave tails (vsb writers) must be emitted
                        # before the first PV pop reads vsb
                        wave_tails(tail_hook)
                        tail_hook = None
                    pt = emit_scores(qi, kk)
                    # invariant: at most one qi's stages (3) may be pending
                    # when a PV is emitted — its psum-ring slots need the
                    # den/yq consumers of qi-2 emitted first
                    while len(deferred) > 3:
                        deferred.popleft()()
                    if len(pv_queue) >= PV_DELAY:
                        finish_pv(*pv_queue.popleft())
                    pv_queue.append((qi, kk, pt))
                    # late q-tiles have short kk loops: drain faster there
                    if deferred and kk >= (2 if qi < 8 else 1):
                        deferred.popleft()()

            att_qi(0, tail_hook=last_wave_items)
            for qi in range(1, NT):
                att_qi(qi)
            while pv_queue:
                while len(deferred) > 3:
                    deferred.popleft()()
                finish_pv(*pv_queue.popleft())
            while deferred:
                deferred.popleft()()

    if dedup:
        n = dedup_ldweights(nc)
        import logging
        logging.getLogger(__name__).info(f"dedup_ldweights removed {n}")
    return nc


def _get_program(T_=T, C_=C, win=WINDOW, dedup=True):
    key = (T_, C_, win, dedup)
    if key not in _PROGRAM_CACHE:
        nc = build_program(T_, C_, win, dedup=dedup)
        nc.finalize()
        _PROGRAM_CACHE[key] = nc
    return _PROGRAM_CACHE[key]


def make_in_maps(x, ve, cos, sin, Wq, Wk, Wv, Wg, Wo):
    """Build the 8 per-core input dicts (host-side sharding/layout prep)."""
    cosT = np.ascontiguousarray(cos[:, 0, :].T).astype(np.float32)  # [64, T]
    sinT = np.ascontiguousarray(sin[:, 0, :].T).astype(np.float32)
    cc = np.concatenate([cosT, cosT], axis=0)            # [128, T]
    ss = np.concatenate([sinT, -sinT], axis=0)           # [128, T]
    # multiplicative 0/1 band masks in S^T coords [kj, q], tiled per head.
    # diag tile (kk==0): keep kj >= q ; far tile (kk==WT): keep kj < q
    kj = np.arange(P)[:, None]
    q = np.arange(P)[None, :]
    m_lo = (kj >= q).astype(np.float32)
    m_hi = (kj < q).astype(np.float32)
    mlo = np.tile(m_lo, (1, GH)).astype(BF16)
    mhi = np.tile(m_hi, (1, GH)).astype(BF16)
    identb = np.eye(P, dtype=np.float32).astype(BF16)

    # partition-major images: element [p, ...] lives on SBUF partition p,
    # so every device DMA is a contiguous per-partition blast
    KT = C // P
    NT = T // P

    def cmaj(a, tile_rows=P):  # [R, N] -> [P, R//P, N]
        r, n = a.shape
        return np.ascontiguousarray(
            a.reshape(r // tile_rows, tile_rows, n).transpose(1, 0, 2))

    in_maps = []
    for core in range(N_CORES):
        b, g = divmod(core, N_KV)
        wq_g = Wq[:, g * GH * HD:(g + 1) * GH * HD]      # [C, GH*HD]
        wq_img = np.ascontiguousarray(
            wq_g.reshape(KT, P, GH, HD).transpose(1, 2, 0, 3))  # [P,GH,KT,HD]
        xT_img = cmaj(np.ascontiguousarray(x[b].T))      # [P, KT, T]
        in_maps.append({
            "xT": xT_img.astype(BF16),
            "wq": wq_img.astype(BF16),
            "wk": cmaj(Wk[:, g * HD:(g + 1) * HD]).astype(BF16),
            "wv": cmaj(Wv[:, g * HD:(g + 1) * HD]).astype(BF16),
            "wg": np.ascontiguousarray(Wg[:, g:g + 1]).astype(BF16),
            "ve2": cmaj(2.0 * ve[b][:, g * HD:(g + 1) * HD]).astype(BF16),
            "wo": cmaj(Wo[g * GH * HD:(g + 1) * GH * HD, :]).astype(BF16),
            "cc": cc.astype(BF16), "ss": ss.astype(BF16),
            "mlo": mlo, "mhi": mhi, "identb": identb,
        })
    return in_maps


def kernel(x, ve, cos, sin, Wq, Wk, Wv, Wg, Wo, window):
    assert int(window) == WINDOW and x.shape == (B, T, C)
    from concourse.bass_utils import run_bass_kernel_spmd

    nc = _get_program()
    in_maps = make_in_maps(x, ve, cos, sin, Wq, Wk, Wv, Wg, Wo)
    res = run_bass_kernel_spmd(nc, in_maps, core_ids=list(range(N_CORES)))
    out = np.zeros((B, T, C), dtype=np.float32)
    for core in range(N_CORES):
        b = core // N_KV
        out[b] += res.results[core]["out"].astype(np.float32)
    return out
